# revision 21
# baseline (speedup 1.0000x reference)
"""Trainium2 Bass kernel for nn_DecoderLayer_66408784331382.

Single transformer decoder layer (RMSNorm + GQA attention w/ RoPE + RMSNorm +
SwiGLU MLP), tensor-parallel over 8 NeuronCores:

  - per core: 4 of 32 Q heads, 1 of 8 KV heads, 1024 of 8192 MLP inter cols,
    matching row-shards of wo / w_down.
  - all on-device activations are kept transposed ([hid, tok] etc.) so that
    every matmul is transpose-free; the host supplies hidden_states.T.
  - RMS statistics use an ACT Square pass + ones-column matmul (partition
    reduction); softmax denominators come from a ones-column appended to V in
    the PV matmul; per-token scaling uses partition-stride-0 broadcast DMAs.
  - one on-device fp32 AllReduce joins attention output partials before the
    second RMSNorm; the final down-proj partials (+ x1/8 each) are summed on
    the host during unsharding.
  - attention path is float32r (full-rate fp32 matmuls); the MLP runs bf16.

kernel(**inputs) takes the FULL fp32 inputs of reference.setup_inputs() and
returns the FULL [1, 2048, 2048] fp32 output.
"""

import sys

if "/opt/trn_rl_repo" not in sys.path:
    sys.path.insert(0, "/opt/trn_rl_repo")

import numpy as np
import ml_dtypes

import concourse.bass as bass
import concourse.mybir as mybir
import concourse.tile as tile
from concourse import bacc
from concourse.bass_utils import run_bass_kernel_spmd

# ---- problem constants (hardcoded per contract) ----
N_CORES = 8
S = 2048
HID = 2048
HD = 64
NH = 32
INTER = 8192
EPS = 1e-6

QD = (NH // N_CORES) * HD        # 256 local q cols
INTER_LOC = INTER // N_CORES     # 1024
SCALE = 1.0 / np.sqrt(HD)

F32 = mybir.dt.float32
F32R = mybir.dt.float32r
BF16 = mybir.dt.bfloat16

P = 128
Q = 512      # phase-1 token quarter
C = 1024     # phase-4 token chunk
ARDT = mybir.dt.float32  # collective dtype
AF = mybir.ActivationFunctionType
ALU = mybir.AluOpType


def _bcast(ap, parts):
    """View a [1, N] AP as [parts, N] via partition-stride-0 (DMA broadcast)."""
    return bass.AP(tensor=ap.tensor, offset=ap.offset,
                   ap=[[0, parts]] + [list(p) for p in ap.ap[1:]])


def build():
    nc = bacc.Bacc("TRN2", target_bir_lowering=False, debug=False,
                   num_devices=N_CORES)

    hT_d = nc.dram_tensor("hT", [HID, S], F32R, kind="ExternalInput")
    sin4_d = nc.dram_tensor("sin4", [P, S], F32R, kind="ExternalInput")
    cos4_d = nc.dram_tensor("cos4", [P, S], F32R, kind="ExternalInput")
    wq_d = nc.dram_tensor("wq", [HID, QD], F32R, kind="ExternalInput")
    wkv_d = nc.dram_tensor("wkv", [HID, 2 * HD], F32R, kind="ExternalInput")
    wo_d = nc.dram_tensor("wo", [QD, HID], F32R, kind="ExternalInput")
    wg_d = nc.dram_tensor("wg", [HID, INTER_LOC], BF16, kind="ExternalInput")
    wu_d = nc.dram_tensor("wu", [HID, INTER_LOC], BF16, kind="ExternalInput")
    wd_d = nc.dram_tensor("wd", [INTER_LOC, HID], BF16, kind="ExternalInput")
    ident_d = nc.dram_tensor("ident", [P, P], F32R, kind="ExternalInput")
    masks_d = nc.dram_tensor("masks", [P, 4 * 512], F32R, kind="ExternalInput")
    outT_d = nc.dram_tensor("outT", [HID, S], F32, kind="ExternalOutput")

    with tile.TileContext(nc) as tc, nc.allow_low_precision(
            reason="float32r is fp32 bits; reciprocal outputs are fp32-width"):
        with (
            tc.tile_pool(name="const", bufs=1) as const,
            tc.tile_pool(name="dramp", bufs=1, space="DRAM") as dram,
        ):
            ones1 = const.tile([P, 1], F32R)
            eps1 = const.tile([P, 1], F32)
            nc.gpsimd.memset(eps1, EPS)
            # f32r memset fails the walrus ISA check; masks[:,0,511] is all-1.0
            nc.sync.dma_start(
                ones1, bass.AP(tensor=masks_d.tensor
                               if hasattr(masks_d, "tensor") else masks_d,
                               offset=511, ap=[[4 * 512, P], [0, 1]]))

            ar_in = [dram.tile([HID, C], ARDT, name=f"ar_in{i}",
                               tag=f"ar_in{i}") for i in range(2)]
            ar_out = [dram.tile([HID, C], ARDT, addr_space="Shared",
                                name=f"ar_out{i}", tag=f"ar_out{i}")
                      for i in range(2)]
            bc1_dram = dram.tile([4, Q], F32R)
            bc2_dram = dram.tile([2, 8, 512], F32R)
            bc4_dram = dram.tile([2, C], F32R)

            # ======== attention scope (phases 1-3 share these tensors) ======
            with tc.tile_pool(name="keep", bufs=1) as keep:
                sin4 = keep.tile([P, S], F32R)
                cos4 = keep.tile([P, S], F32R)
                ident = keep.tile([P, P], F32R)
                masks = keep.tile([P, 4, 512], F32R)
                nc.sync.dma_start(sin4, sin4_d[:, :])
                nc.sync.dma_start(cos4, cos4_d[:, :])
                nc.sync.dma_start(ident, ident_d[:, :])
                nc.sync.dma_start(
                    masks, masks_d[:, :].rearrange("p (t n) -> p t n", t=4))
                qT = [keep.tile([P, S], F32R, tag=f"qT{m}", name=f"qT{m}") for m in range(2)]
                kTdup = keep.tile([P, S], F32R, tag="kTdup")
                v_ones = keep.tile([P, 16, HD + 1], F32R, tag="v_ones")
                attnT = [keep.tile([P, S], F32R, tag=f"attnT{m}", name=f"attnT{m}")
                         for m in range(2)]
                nc.sync.dma_start(
                    v_ones[:, :, HD:HD + 1],
                    bass.AP(tensor=masks_d.tensor
                            if hasattr(masks_d, "tensor") else masks_d,
                            offset=511, ap=[[4 * 512, P], [0, 16], [0, 1]]))

                # ---- Phase 1: RMS1 + QKV + RoPE, per 512-token quarter ----
                with (
                    tc.tile_pool(name="p1w", bufs=1) as p1w,
                    tc.tile_pool(name="p1x", bufs=1) as p1x,
                    tc.tile_pool(name="p1s", bufs=1) as p1s,
                    tc.tile_pool(name="p1ps", bufs=2, space="PSUM") as p1ps,
                    tc.tile_pool(name="p1ps_s", bufs=1, space="PSUM") as p1pss,
                ):
                    wq_all = p1w.tile([P, 16, QD], F32R)
                    wkv_all = p1w.tile([P, 16, 2 * HD], F32R)
                    nc.sync.dma_start(
                        wq_all, wq_d[:, :].rearrange("(t p) m -> p t m", p=P))
                    nc.sync.dma_start(
                        wkv_all, wkv_d[:, :].rearrange("(t p) m -> p t m", p=P))
                    xn1 = p1x.tile([P, 16, Q], F32R, tag="xn1", bufs=2)

                    for q4 in range(4):
                        qc = slice(Q * q4, Q * (q4 + 1))
                        # RMS1 stats
                        ssq = p1pss.tile([1, Q], F32, tag="ssq")
                        for t4 in range(4):
                            nc.sync.dma_start(
                                xn1[:, 4 * t4:4 * (t4 + 1), :],
                                hT_d[512 * t4:512 * (t4 + 1), qc].rearrange(
                                    "(t p) m -> p t m", p=P))
                        for kt in range(16):
                            xt = xn1[:, kt, :]
                            sq = p1s.tile([P, Q], F32R, tag="sq", bufs=3)
                            nc.scalar.activation(sq, xt, AF.Square)
                            nc.tensor.matmul(ssq, ones1, sq,
                                             start=(kt == 0), stop=(kt == 15))
                        rms = p1s.tile([1, Q], F32R, tag="rms", bufs=2)
                        nc.scalar.activation(rms, ssq, AF.Sqrt,
                                             bias=eps1[0:1, :], scale=1.0 / HID)
                        inv = p1s.tile([1, Q], F32R, tag="inv", bufs=2)
                        nc.vector.reciprocal(inv, rms)
                        invb = p1s.tile([P, Q], F32R, tag="invb", bufs=2)
                        nc.sync.dma_start(bc1_dram[q4:q4 + 1, :], inv)
                        nc.sync.dma_start(invb, _bcast(bc1_dram[q4:q4 + 1, :], P))
                        for kt in range(16):
                            nc.vector.tensor_mul(xn1[:, kt, :],
                                                 xn1[:, kt, :], invb)

                        # QKV projections (transposed outputs)
                        q_ps = [p1ps.tile([P, Q], F32, tag=f"qps{m}", name=f"qps{m}")
                                for m in range(2)]
                        kv_ps = p1ps.tile([P, Q], F32, tag="kvps")
                        for kt in range(16):
                            st, sp = (kt == 0), (kt == 15)
                            for m in range(2):
                                nc.tensor.matmul(
                                    q_ps[m], wq_all[:, kt, P * m:P * (m + 1)],
                                    xn1[:, kt, :], start=st, stop=sp)
                            nc.tensor.matmul(kv_ps, wkv_all[:, kt, :],
                                             xn1[:, kt, :], start=st, stop=sp)

                        # RoPE eviction (sin4 rows carry the rotate-half
                        # sign: +sinT for x0, -sinT for x1 source rows):
                        # out = ps*cos + swap_half(ps)*sinA
                        for m in range(2):
                            s1 = p1s.tile([P, Q], F32R, tag="s1", bufs=2)
                            s2 = p1s.tile([P, Q], F32R, tag="s2", bufs=2)
                            nc.vector.tensor_mul(s1, q_ps[m], cos4[:, qc])
                            for b in range(2):
                                x0 = slice(64 * b, 64 * b + 32)
                                x1s = slice(64 * b + 32, 64 * b + 64)
                                nc.vector.tensor_mul(
                                    s2[x0, :], q_ps[m][x1s, :], sin4[x1s, qc])
                                nc.vector.tensor_mul(
                                    s2[x1s, :], q_ps[m][x0, :], sin4[x0, qc])
                            nc.vector.tensor_add(qT[m][:, qc], s1, s2)
                        # RoPE eviction: k, duplicated into rows 64:128
                        s1 = p1s.tile([64, Q], F32R, tag="s1k", bufs=2)
                        s2 = p1s.tile([64, Q], F32R, tag="s2k", bufs=2)
                        nc.vector.tensor_mul(s1, kv_ps[0:64, :], cos4[0:64, qc])
                        nc.vector.tensor_mul(
                            s2[0:32, :], kv_ps[32:64, :], sin4[32:64, qc])
                        nc.vector.tensor_mul(
                            s2[32:64, :], kv_ps[0:32, :], sin4[0:32, qc])
                        nc.vector.tensor_add(kTdup[0:64, qc], s1, s2)
                        nc.vector.tensor_copy(kTdup[64:128, qc], kTdup[0:64, qc])
                        # v: vT then PE-transpose into v_ones
                        vt = p1s.tile([64, Q], F32R, tag="vt", bufs=2)
                        nc.vector.tensor_copy(vt, kv_ps[64:128, :])
                        for j in range(4):
                            ktg = 4 * q4 + j
                            vtp = p1pss.tile([P, HD], F32R, tag="vtp")
                            nc.tensor.transpose(
                                vtp, vt[:, P * j:P * (j + 1)],
                                ident[0:64, 0:64])
                            nc.vector.tensor_copy(v_ones[:, ktg, 0:HD], vtp)

                # ---- Phase 2: scoresT -> exp/mask -> PV (+denominator) ----
                with (
                    tc.tile_pool(name="p2pr", bufs=3) as p2pr,
                    tc.tile_pool(name="p2sm", bufs=2) as p2sm,
                    tc.tile_pool(name="p2ps", bufs=2, space="PSUM") as p2ps,
                    tc.tile_pool(name="p2pv", bufs=2, space="PSUM") as p2pv,
                ):
                    for qc4 in range(4):
                        for m in range(2):
                            qs = slice(512 * qc4, 512 * (qc4 + 1))
                            pv = [p2pv.tile([HD + 1, 512], F32, tag=f"pv{b}", name=f"pv{b}")
                                  for b in range(2)]
                            nkt = 4 * qc4 + 4
                            for kt in range(nkt):
                                st, sp = (kt == 0), (kt == nkt - 1)
                                for b in range(2):
                                    rows = slice(64 * b, 64 * (b + 1))
                                    sc = p2ps.tile([P, 512], F32, tag=f"sc{b}")
                                    nc.tensor.matmul(
                                        sc, kTdup[rows, P * kt:P * (kt + 1)],
                                        qT[m][rows, qs], start=True, stop=True)
                                    pr = p2pr.tile([P, 512], F32R, tag=f"pr{b}")
                                    nc.scalar.activation(pr, sc, AF.Exp,
                                                         scale=float(SCALE))
                                    if kt >= 4 * qc4:
                                        nc.vector.tensor_mul(
                                            pr, pr, masks[:, kt - 4 * qc4, :])
                                    nc.tensor.matmul(pv[b], v_ones[:, kt, :],
                                                     pr, start=st, stop=sp)
                            for b in range(2):
                                rec = p2sm.tile([1, 512], F32R, tag=f"rec{b}")
                                nc.vector.reciprocal(rec, pv[b][HD:HD + 1, :])
                                slot = bc2_dram[b:b + 1, 4 * m + qc4, :]
                                nc.sync.dma_start(slot, rec)
                                recb = p2sm.tile([64, 512], F32R,
                                                 tag=f"recb{b}")
                                nc.sync.dma_start(recb, _bcast(slot, 64))
                                nc.vector.tensor_mul(
                                    attnT[m][64 * b:64 * (b + 1), qs],
                                    pv[b][0:HD, :], recb)

                # ---- Phase 3: o-proj partials -> ar_in ----
                with (
                    tc.tile_pool(name="p3w", bufs=1) as p3w,
                    tc.tile_pool(name="p3o", bufs=3) as p3o,
                    tc.tile_pool(name="p3ps", bufs=3, space="PSUM") as p3ps,
                ):
                    wo_all = p3w.tile([P, 2, HID], F32R)
                    nc.sync.dma_start(
                        wo_all, wo_d[:, :].rearrange("(t p) m -> p t m", p=P))
                    for c3 in range(2):
                        for hm in range(16):
                            osb = p3o.tile([P, C], ARDT, tag="osb")
                            for nq in range(2):
                                qc4 = 2 * c3 + nq
                                qs = slice(512 * qc4, 512 * (qc4 + 1))
                                ops = p3ps.tile([P, 512], F32, tag="ops")
                                for kt2 in range(2):
                                    nc.tensor.matmul(
                                        ops,
                                        wo_all[:, kt2, P * hm:P * (hm + 1)],
                                        attnT[kt2][:, qs],
                                        start=(kt2 == 0), stop=(kt2 == 1))
                                nc.scalar.copy(
                                    osb[:, 512 * nq:512 * (nq + 1)], ops)
                            nc.sync.dma_start(
                                ar_in[c3][P * hm:P * (hm + 1), :], osb)
                        # AllReduce for this token half (overlaps what follows)
                        nc.gpsimd.collective_compute(
                            "AllReduce", ALU.add,
                            replica_groups=[list(range(N_CORES))],
                            ins=[ar_in[c3][:, :].opt()],
                            outs=[ar_out[c3][:, :].opt()])

            # ---- Phase 4: x1 + RMS2 + SwiGLU MLP, per 1024-token chunk ----
            with (
                tc.tile_pool(name="p4x", bufs=1) as p4x,
                tc.tile_pool(name="p4s", bufs=1) as p4s,
                tc.tile_pool(name="p4w", bufs=1) as p4w,
                tc.tile_pool(name="p4ps_s", bufs=1, space="PSUM") as p4pss,
                tc.tile_pool(name="p4ps_gu", bufs=1, space="PSUM") as p4gu,
                tc.tile_pool(name="p4ps_d", bufs=1, space="PSUM") as p4d,
            ):
                x1 = p4x.tile([P, 16, C], F32R, tag="x1")
                xn2 = p4x.tile([P, 16, C], BF16, tag="xn2")
                hmlp = p4x.tile([P, 8, C], BF16, tag="hmlp")
                for c2 in range(2):
                    cc = slice(C * c2, C * (c2 + 1))
                    # x1 = hidden + attn_out ; RMS2 stats
                    ssq2 = p4pss.tile([1, C], F32, tag="ssq2")
                    for kt in range(16):
                        rs = slice(P * kt, P * (kt + 1))
                        th = p4s.tile([P, C], F32R, tag="th", bufs=2)
                        ta = p4s.tile([P, C], ARDT, tag="ta", bufs=2)
                        nc.sync.dma_start(th, hT_d[rs, cc])
                        nc.sync.dma_start(ta, ar_out[c2][rs, :])
                        nc.vector.tensor_add(x1[:, kt, :], th, ta)
                        sq = p4s.tile([P, C], F32R, tag="sq2", bufs=2)
                        nc.scalar.activation(sq, x1[:, kt, :], AF.Square)
                        for n in range(2):
                            nc.tensor.matmul(
                                ssq2[:, 512 * n:512 * (n + 1)], ones1,
                                sq[:, 512 * n:512 * (n + 1)],
                                start=(kt == 0), stop=(kt == 15))
                    rms = p4s.tile([1, C], F32R, tag="rms2", bufs=2)
                    nc.scalar.activation(rms, ssq2, AF.Sqrt,
                                         bias=eps1[0:1, :], scale=1.0 / HID)
                    inv = p4s.tile([1, C], F32R, tag="inv2", bufs=2)
                    nc.vector.reciprocal(inv, rms)
                    invb = p4s.tile([P, C], F32R, tag="invb2", bufs=1)
                    nc.sync.dma_start(bc4_dram[c2:c2 + 1, :], inv)
                    nc.sync.dma_start(invb, _bcast(bc4_dram[c2:c2 + 1, :], P))
                    for kt in range(16):
                        nc.vector.tensor_mul(xn2[:, kt, :], x1[:, kt, :], invb)

                    # gate/up + silu*up (bf16)
                    for iq in range(8):
                        gps = p4gu.tile([P, C], F32, tag="g")
                        ups = p4gu.tile([P, C], F32, tag="u")
                        wg_t = p4w.tile([P, 16, P], BF16, tag="wgt", bufs=2)
                        wu_t = p4w.tile([P, 16, P], BF16, tag="wut", bufs=2)
                        nc.sync.dma_start(
                            wg_t, wg_d[:, P * iq:P * (iq + 1)].rearrange(
                                "(t p) m -> p t m", p=P))
                        nc.sync.dma_start(
                            wu_t, wu_d[:, P * iq:P * (iq + 1)].rearrange(
                                "(t p) m -> p t m", p=P))
                        for kt in range(16):
                            st, sp = (kt == 0), (kt == 15)
                            for n in range(2):
                                ns = slice(512 * n, 512 * (n + 1))
                                nc.tensor.matmul(gps[:, ns], wg_t[:, kt, :],
                                                 xn2[:, kt, ns],
                                                 start=st, stop=sp)
                                nc.tensor.matmul(ups[:, ns], wu_t[:, kt, :],
                                                 xn2[:, kt, ns],
                                                 start=st, stop=sp)
                        sg = p4s.tile([P, C], BF16, tag="sg", bufs=2)
                        nc.scalar.activation(sg, gps, AF.Silu)
                        nc.vector.tensor_mul(hmlp[:, iq, :], sg, ups)

                    # down proj + (x1/8) residual share -> outT
                    for hm in range(16):
                        dps = p4d.tile([P, C], F32, tag="d")
                        wd_t = p4w.tile([P, 8, P], BF16, tag="wdt", bufs=2)
                        nc.sync.dma_start(
                            wd_t, wd_d[:, P * hm:P * (hm + 1)].rearrange(
                                "(t p) m -> p t m", p=P))
                        for kt8 in range(8):
                            for n in range(2):
                                ns = slice(512 * n, 512 * (n + 1))
                                nc.tensor.matmul(dps[:, ns], wd_t[:, kt8, :],
                                                 hmlp[:, kt8, ns],
                                                 start=(kt8 == 0),
                                                 stop=(kt8 == 7))
                        dsb = p4s.tile([P, C], F32, tag="dsb", bufs=2)
                        nc.vector.scalar_tensor_tensor(
                            dsb, x1[:, hm, :], 1.0 / N_CORES, dps,
                            op0=ALU.mult, op1=ALU.add)
                        nc.sync.dma_start(outT_d[P * hm:P * (hm + 1), cc], dsb)

    nc.compile()
    return nc


_CACHE = {}


def _get_nc():
    if "nc" not in _CACHE:
        _CACHE["nc"] = build()
    return _CACHE["nc"]


def _prep_inputs(inputs):
    """Shard + preprocess full inputs into 8 per-core in_maps."""
    f = lambda k: np.asarray(inputs[k], dtype=np.float32)
    hidden = f("hidden_states")[0]                 # [S, HID]
    sin_t, cos_t = f("sin_table"), f("cos_table")  # [S, 32]
    ln1, ln2 = f("ln1_w"), f("ln2_w")
    wq = f("wq") * ln1[:, None]
    wk = f("wk") * ln1[:, None]
    wv = f("wv") * ln1[:, None]
    wo = f("wo")
    wg = (f("w_gate") * ln2[:, None]).astype(ml_dtypes.bfloat16)
    wu = (f("w_up") * ln2[:, None]).astype(ml_dtypes.bfloat16)
    wd = f("w_down").astype(ml_dtypes.bfloat16)

    hT = np.ascontiguousarray(hidden.T)
    # rows per 64-block: [+sinT (x0 source); -sinT (x1 source)]
    sin4 = np.ascontiguousarray(
        np.tile(np.concatenate([sin_t.T, -sin_t.T], axis=0), (2, 1)))
    cos4 = np.ascontiguousarray(np.tile(cos_t.T, (4, 1)))
    ident = np.eye(P, dtype=np.float32)
    rr = np.arange(P)[:, None]
    cols = np.arange(512)[None, :]
    masks = np.concatenate(
        [(rr + 128 * t <= cols).astype(np.float32) for t in range(4)],
        axis=1)

    in_maps = []
    for c in range(N_CORES):
        qs = slice(QD * c, QD * (c + 1))
        ks = slice(HD * c, HD * (c + 1))
        isl = slice(INTER_LOC * c, INTER_LOC * (c + 1))
        in_maps.append({
            "hT": hT,
            "sin4": sin4,
            "cos4": cos4,
            "wq": np.ascontiguousarray(wq[:, qs]),
            "wkv": np.ascontiguousarray(
                np.concatenate([wk[:, ks], wv[:, ks]], axis=1)),
            "wo": np.ascontiguousarray(wo[qs, :]),
            "wg": np.ascontiguousarray(wg[:, isl]),
            "wu": np.ascontiguousarray(wu[:, isl]),
            "wd": np.ascontiguousarray(wd[isl, :]),
            "ident": ident,
            "masks": masks,
        })
    return in_maps


def kernel(**inputs):
    nc = _get_nc()
    in_maps = _prep_inputs(inputs)
    res = run_bass_kernel_spmd(nc, in_maps, core_ids=list(range(N_CORES)))
    acc = np.zeros((HID, S), dtype=np.float32)
    for c in range(N_CORES):
        acc += res.results[c]["outT"]
    return np.ascontiguousarray(acc.T)[None, :, :]


# revision 22
# speedup vs baseline: 1.0156x; 1.0156x over previous
"""Trainium2 Bass kernel for nn_DecoderLayer_66408784331382.

Single transformer decoder layer (RMSNorm + GQA attention w/ RoPE + RMSNorm +
SwiGLU MLP), tensor-parallel over 8 NeuronCores:

  - per core: 4 of 32 Q heads, 1 of 8 KV heads, 1024 of 8192 MLP inter cols,
    matching row-shards of wo / w_down.
  - all on-device activations are kept transposed ([hid, tok] etc.) so that
    every matmul is transpose-free; the host supplies hidden_states.T.
  - RMS statistics use an ACT Square pass + ones-column matmul (partition
    reduction); softmax denominators come from a ones-column appended to V in
    the PV matmul; per-token scaling uses partition-stride-0 broadcast DMAs.
  - one on-device fp32 AllReduce joins attention output partials before the
    second RMSNorm; the final down-proj partials (+ x1/8 each) are summed on
    the host during unsharding.
  - attention path is float32r (full-rate fp32 matmuls); the MLP runs bf16.

kernel(**inputs) takes the FULL fp32 inputs of reference.setup_inputs() and
returns the FULL [1, 2048, 2048] fp32 output.
"""

import sys

if "/opt/trn_rl_repo" not in sys.path:
    sys.path.insert(0, "/opt/trn_rl_repo")

import numpy as np
import ml_dtypes

import concourse.bass as bass
import concourse.mybir as mybir
import concourse.tile as tile
from concourse import bacc
from concourse.bass_utils import run_bass_kernel_spmd

# ---- problem constants (hardcoded per contract) ----
N_CORES = 8
S = 2048
HID = 2048
HD = 64
NH = 32
INTER = 8192
EPS = 1e-6

QD = (NH // N_CORES) * HD        # 256 local q cols
INTER_LOC = INTER // N_CORES     # 1024
SCALE = 1.0 / np.sqrt(HD)

F32 = mybir.dt.float32
F32R = mybir.dt.float32r
BF16 = mybir.dt.bfloat16

P = 128
Q = 512      # phase-1 token quarter
C = 1024     # phase-4 token chunk
ARDT = mybir.dt.float32  # collective dtype
AF = mybir.ActivationFunctionType
ALU = mybir.AluOpType


def _bcast(ap, parts):
    """View a [1, N] AP as [parts, N] via partition-stride-0 (DMA broadcast)."""
    return bass.AP(tensor=ap.tensor, offset=ap.offset,
                   ap=[[0, parts]] + [list(p) for p in ap.ap[1:]])


def build():
    nc = bacc.Bacc("TRN2", target_bir_lowering=False, debug=False,
                   num_devices=N_CORES)

    hT_d = nc.dram_tensor("hT", [HID, S], F32R, kind="ExternalInput")
    sin4_d = nc.dram_tensor("sin4", [P, S], F32R, kind="ExternalInput")
    cos4_d = nc.dram_tensor("cos4", [P, S], F32R, kind="ExternalInput")
    wq_d = nc.dram_tensor("wq", [HID, QD], F32R, kind="ExternalInput")
    wkv_d = nc.dram_tensor("wkv", [HID, 2 * HD], F32R, kind="ExternalInput")
    wo_d = nc.dram_tensor("wo", [QD, HID], F32R, kind="ExternalInput")
    wg_d = nc.dram_tensor("wg", [HID, INTER_LOC], BF16, kind="ExternalInput")
    wu_d = nc.dram_tensor("wu", [HID, INTER_LOC], BF16, kind="ExternalInput")
    wd_d = nc.dram_tensor("wd", [INTER_LOC, HID], BF16, kind="ExternalInput")
    ident_d = nc.dram_tensor("ident", [P, P], F32R, kind="ExternalInput")
    masks_d = nc.dram_tensor("masks", [P, 4 * 512], F32R, kind="ExternalInput")
    outT_d = nc.dram_tensor("outT", [HID, S], F32, kind="ExternalOutput")

    with tile.TileContext(nc) as tc, nc.allow_low_precision(
            reason="float32r is fp32 bits; reciprocal outputs are fp32-width"):
        with (
            tc.tile_pool(name="const", bufs=1) as const,
            tc.tile_pool(name="dramp", bufs=1, space="DRAM") as dram,
        ):
            ones1 = const.tile([P, 1], F32R)
            eps1 = const.tile([P, 1], F32)
            nc.gpsimd.memset(eps1, EPS)
            # f32r memset fails the walrus ISA check; masks[:,0,511] is all-1.0
            nc.sync.dma_start(
                ones1, bass.AP(tensor=masks_d.tensor
                               if hasattr(masks_d, "tensor") else masks_d,
                               offset=511, ap=[[4 * 512, P], [0, 1]]))

            ar_in = [dram.tile([HID, C], ARDT, name=f"ar_in{i}",
                               tag=f"ar_in{i}") for i in range(2)]
            ar_out = [dram.tile([HID, C], ARDT, addr_space="Shared",
                                name=f"ar_out{i}", tag=f"ar_out{i}")
                      for i in range(2)]
            bc1_dram = dram.tile([4, Q], F32R)
            bc2_dram = dram.tile([2, 8, 512], F32R)
            bc4_dram = dram.tile([2, C], F32R)

            # ======== attention scope (phases 1-3 share these tensors) ======
            with tc.tile_pool(name="keep", bufs=1) as keep:
                sin4 = keep.tile([P, S], F32R)
                cos4 = keep.tile([P, S], F32R)
                ident = keep.tile([P, P], F32R)
                masks = keep.tile([P, 4, 512], F32R)
                nc.sync.dma_start(sin4, sin4_d[:, :])
                nc.sync.dma_start(cos4, cos4_d[:, :])
                nc.sync.dma_start(ident, ident_d[:, :])
                nc.sync.dma_start(
                    masks, masks_d[:, :].rearrange("p (t n) -> p t n", t=4))
                qT = [keep.tile([P, S], F32R, tag=f"qT{m}", name=f"qT{m}") for m in range(2)]
                kTdup = keep.tile([P, S], F32R, tag="kTdup")
                v_ones = keep.tile([P, 16, HD + 1], F32R, tag="v_ones")
                attnT = [keep.tile([P, S], F32R, tag=f"attnT{m}", name=f"attnT{m}")
                         for m in range(2)]
                nc.sync.dma_start(
                    v_ones[:, :, HD:HD + 1],
                    bass.AP(tensor=masks_d.tensor
                            if hasattr(masks_d, "tensor") else masks_d,
                            offset=511, ap=[[4 * 512, P], [0, 16], [0, 1]]))

                # ---- Phase 1: RMS1 + QKV + RoPE, per 512-token quarter ----
                with (
                    tc.tile_pool(name="p1w", bufs=1) as p1w,
                    tc.tile_pool(name="p1x", bufs=1) as p1x,
                    tc.tile_pool(name="p1s", bufs=1) as p1s,
                    tc.tile_pool(name="p1ps", bufs=2, space="PSUM") as p1ps,
                    tc.tile_pool(name="p1ps_s", bufs=1, space="PSUM") as p1pss,
                ):
                    wq_all = p1w.tile([P, 16, QD], F32R)
                    wkv_all = p1w.tile([P, 16, 2 * HD], F32R)
                    nc.sync.dma_start(
                        wq_all, wq_d[:, :].rearrange("(t p) m -> p t m", p=P))
                    nc.sync.dma_start(
                        wkv_all, wkv_d[:, :].rearrange("(t p) m -> p t m", p=P))
                    xn1 = p1x.tile([P, 16, Q], F32R, tag="xn1")

                    for q4 in range(4):
                        qc = slice(Q * q4, Q * (q4 + 1))
                        # RMS1 stats
                        ssq = p1pss.tile([1, Q], F32, tag="ssq")
                        for t4 in range(4):
                            nc.sync.dma_start(
                                xn1[:, 4 * t4:4 * (t4 + 1), :],
                                hT_d[512 * t4:512 * (t4 + 1), qc].rearrange(
                                    "(t p) m -> p t m", p=P))
                        for kt in range(16):
                            xt = xn1[:, kt, :]
                            sq = p1s.tile([P, Q], F32R, tag="sq", bufs=3)
                            nc.scalar.activation(sq, xt, AF.Square)
                            nc.tensor.matmul(ssq, ones1, sq,
                                             start=(kt == 0), stop=(kt == 15))
                        rms = p1s.tile([1, Q], F32R, tag="rms", bufs=2)
                        nc.scalar.activation(rms, ssq, AF.Sqrt,
                                             bias=eps1[0:1, :], scale=1.0 / HID)
                        inv = p1s.tile([1, Q], F32R, tag="inv", bufs=2)
                        nc.vector.reciprocal(inv, rms)
                        invb = p1s.tile([P, Q], F32R, tag="invb", bufs=2)
                        nc.sync.dma_start(bc1_dram[q4:q4 + 1, :], inv)
                        nc.sync.dma_start(invb, _bcast(bc1_dram[q4:q4 + 1, :], P))
                        for kt in range(16):
                            nc.vector.tensor_mul(xn1[:, kt, :],
                                                 xn1[:, kt, :], invb)

                        # QKV projections (transposed outputs)
                        q_ps = [p1ps.tile([P, Q], F32, tag=f"qps{m}", name=f"qps{m}")
                                for m in range(2)]
                        kv_ps = p1ps.tile([P, Q], F32, tag="kvps")
                        for kt in range(16):
                            st, sp = (kt == 0), (kt == 15)
                            for m in range(2):
                                nc.tensor.matmul(
                                    q_ps[m], wq_all[:, kt, P * m:P * (m + 1)],
                                    xn1[:, kt, :], start=st, stop=sp)
                            nc.tensor.matmul(kv_ps, wkv_all[:, kt, :],
                                             xn1[:, kt, :], start=st, stop=sp)

                        # RoPE eviction (sin4 rows carry the rotate-half
                        # sign: +sinT for x0, -sinT for x1 source rows):
                        # out = ps*cos + swap_half(ps)*sinA
                        for m in range(2):
                            s1 = p1s.tile([P, Q], F32R, tag="s1", bufs=2)
                            s2 = p1s.tile([P, Q], F32R, tag="s2", bufs=2)
                            nc.vector.tensor_mul(s1, q_ps[m], cos4[:, qc])
                            for b in range(2):
                                x0 = slice(64 * b, 64 * b + 32)
                                x1s = slice(64 * b + 32, 64 * b + 64)
                                nc.vector.tensor_mul(
                                    s2[x0, :], q_ps[m][x1s, :], sin4[x1s, qc])
                                nc.vector.tensor_mul(
                                    s2[x1s, :], q_ps[m][x0, :], sin4[x0, qc])
                            nc.vector.tensor_add(qT[m][:, qc], s1, s2)
                        # RoPE eviction: k, duplicated into rows 64:128
                        s1 = p1s.tile([64, Q], F32R, tag="s1k", bufs=2)
                        s2 = p1s.tile([64, Q], F32R, tag="s2k", bufs=2)
                        nc.vector.tensor_mul(s1, kv_ps[0:64, :], cos4[0:64, qc])
                        nc.vector.tensor_mul(
                            s2[0:32, :], kv_ps[32:64, :], sin4[32:64, qc])
                        nc.vector.tensor_mul(
                            s2[32:64, :], kv_ps[0:32, :], sin4[0:32, qc])
                        nc.vector.tensor_add(kTdup[0:64, qc], s1, s2)
                        nc.vector.tensor_copy(kTdup[64:128, qc], kTdup[0:64, qc])
                        # v: vT then PE-transpose into v_ones
                        vt = p1s.tile([64, Q], F32R, tag="vt", bufs=2)
                        nc.vector.tensor_copy(vt, kv_ps[64:128, :])
                        for j in range(4):
                            ktg = 4 * q4 + j
                            vtp = p1pss.tile([P, HD], F32R, tag="vtp")
                            nc.tensor.transpose(
                                vtp, vt[:, P * j:P * (j + 1)],
                                ident[0:64, 0:64])
                            nc.vector.tensor_copy(v_ones[:, ktg, 0:HD], vtp)

                # ---- Phase 2: scoresT -> exp/mask -> PV (+denominator) ----
                with (
                    tc.tile_pool(name="p2pr", bufs=3) as p2pr,
                    tc.tile_pool(name="p2sm", bufs=2) as p2sm,
                    tc.tile_pool(name="p2ps", bufs=2, space="PSUM") as p2ps,
                    tc.tile_pool(name="p2pv", bufs=2, space="PSUM") as p2pv,
                ):
                    for qc4 in range(4):
                        for m in range(2):
                            qs = slice(512 * qc4, 512 * (qc4 + 1))
                            pv = [p2pv.tile([HD + 1, 512], F32, tag=f"pv{b}", name=f"pv{b}")
                                  for b in range(2)]
                            nkt = 4 * qc4 + 4
                            for kt in range(nkt):
                                st, sp = (kt == 0), (kt == nkt - 1)
                                for b in range(2):
                                    rows = slice(64 * b, 64 * (b + 1))
                                    sc = p2ps.tile([P, 512], F32, tag=f"sc{b}")
                                    nc.tensor.matmul(
                                        sc, kTdup[rows, P * kt:P * (kt + 1)],
                                        qT[m][rows, qs], start=True, stop=True)
                                    pr = p2pr.tile([P, 512], F32R, tag=f"pr{b}")
                                    nc.scalar.activation(pr, sc, AF.Exp,
                                                         scale=float(SCALE))
                                    if kt >= 4 * qc4:
                                        nc.vector.tensor_mul(
                                            pr, pr, masks[:, kt - 4 * qc4, :])
                                    nc.tensor.matmul(pv[b], v_ones[:, kt, :],
                                                     pr, start=st, stop=sp)
                            for b in range(2):
                                rec = p2sm.tile([1, 512], F32R, tag=f"rec{b}")
                                nc.vector.reciprocal(rec, pv[b][HD:HD + 1, :])
                                slot = bc2_dram[b:b + 1, 4 * m + qc4, :]
                                nc.sync.dma_start(slot, rec)
                                recb = p2sm.tile([64, 512], F32R,
                                                 tag=f"recb{b}")
                                nc.sync.dma_start(recb, _bcast(slot, 64))
                                nc.vector.tensor_mul(
                                    attnT[m][64 * b:64 * (b + 1), qs],
                                    pv[b][0:HD, :], recb)

                # ---- Phase 3: o-proj partials -> ar_in ----
                with (
                    tc.tile_pool(name="p3w", bufs=1) as p3w,
                    tc.tile_pool(name="p3o", bufs=3) as p3o,
                    tc.tile_pool(name="p3ps", bufs=3, space="PSUM") as p3ps,
                ):
                    wo_all = p3w.tile([P, 2, HID], F32R)
                    nc.sync.dma_start(
                        wo_all, wo_d[:, :].rearrange("(t p) m -> p t m", p=P))
                    for c3 in range(2):
                        for hm in range(16):
                            osb = p3o.tile([P, C], ARDT, tag="osb")
                            for nq in range(2):
                                qc4 = 2 * c3 + nq
                                qs = slice(512 * qc4, 512 * (qc4 + 1))
                                ops = p3ps.tile([P, 512], F32, tag="ops")
                                for kt2 in range(2):
                                    nc.tensor.matmul(
                                        ops,
                                        wo_all[:, kt2, P * hm:P * (hm + 1)],
                                        attnT[kt2][:, qs],
                                        start=(kt2 == 0), stop=(kt2 == 1))
                                nc.scalar.copy(
                                    osb[:, 512 * nq:512 * (nq + 1)], ops)
                            nc.sync.dma_start(
                                ar_in[c3][P * hm:P * (hm + 1), :], osb)
                        # AllReduce for this token half (overlaps what follows)
                        nc.gpsimd.collective_compute(
                            "AllReduce", ALU.add,
                            replica_groups=[list(range(N_CORES))],
                            ins=[ar_in[c3][:, :].opt()],
                            outs=[ar_out[c3][:, :].opt()])

            # ---- Phase 4: x1 + RMS2 + SwiGLU MLP, per 1024-token chunk ----
            with (
                tc.tile_pool(name="p4x", bufs=1) as p4x,
                tc.tile_pool(name="p4s", bufs=1) as p4s,
                tc.tile_pool(name="p4w", bufs=1) as p4w,
                tc.tile_pool(name="p4ps_s", bufs=1, space="PSUM") as p4pss,
                tc.tile_pool(name="p4ps_gu", bufs=1, space="PSUM") as p4gu,
                tc.tile_pool(name="p4ps_d", bufs=1, space="PSUM") as p4d,
            ):
                x1 = p4x.tile([P, 16, C], F32R, tag="x1")
                xn2 = p4x.tile([P, 16, C], BF16, tag="xn2")
                hmlp = p4x.tile([P, 8, C], BF16, tag="hmlp")
                for c2 in range(2):
                    cc = slice(C * c2, C * (c2 + 1))
                    # x1 = hidden + attn_out ; RMS2 stats
                    ssq2 = p4pss.tile([1, C], F32, tag="ssq2")
                    for kt in range(16):
                        rs = slice(P * kt, P * (kt + 1))
                        th = p4s.tile([P, C], F32R, tag="th", bufs=2)
                        ta = p4s.tile([P, C], ARDT, tag="ta", bufs=2)
                        nc.sync.dma_start(th, hT_d[rs, cc])
                        nc.sync.dma_start(ta, ar_out[c2][rs, :])
                        nc.vector.tensor_add(x1[:, kt, :], th, ta)
                        sq = p4s.tile([P, C], F32R, tag="sq2", bufs=2)
                        nc.scalar.activation(sq, x1[:, kt, :], AF.Square)
                        for n in range(2):
                            nc.tensor.matmul(
                                ssq2[:, 512 * n:512 * (n + 1)], ones1,
                                sq[:, 512 * n:512 * (n + 1)],
                                start=(kt == 0), stop=(kt == 15))
                    rms = p4s.tile([1, C], F32R, tag="rms2", bufs=2)
                    nc.scalar.activation(rms, ssq2, AF.Sqrt,
                                         bias=eps1[0:1, :], scale=1.0 / HID)
                    inv = p4s.tile([1, C], F32R, tag="inv2", bufs=2)
                    nc.vector.reciprocal(inv, rms)
                    invb = p4s.tile([P, C], F32R, tag="invb2", bufs=1)
                    nc.sync.dma_start(bc4_dram[c2:c2 + 1, :], inv)
                    nc.sync.dma_start(invb, _bcast(bc4_dram[c2:c2 + 1, :], P))
                    for kt in range(16):
                        nc.vector.tensor_mul(xn2[:, kt, :], x1[:, kt, :], invb)

                    # gate/up + silu*up (bf16)
                    for iq in range(8):
                        gps = p4gu.tile([P, C], F32, tag="g")
                        ups = p4gu.tile([P, C], F32, tag="u")
                        wg_t = p4w.tile([P, 16, P], BF16, tag="wgt", bufs=2)
                        wu_t = p4w.tile([P, 16, P], BF16, tag="wut", bufs=2)
                        nc.sync.dma_start(
                            wg_t, wg_d[:, P * iq:P * (iq + 1)].rearrange(
                                "(t p) m -> p t m", p=P))
                        nc.sync.dma_start(
                            wu_t, wu_d[:, P * iq:P * (iq + 1)].rearrange(
                                "(t p) m -> p t m", p=P))
                        for kt in range(16):
                            st, sp = (kt == 0), (kt == 15)
                            for n in range(2):
                                ns = slice(512 * n, 512 * (n + 1))
                                nc.tensor.matmul(gps[:, ns], wg_t[:, kt, :],
                                                 xn2[:, kt, ns],
                                                 start=st, stop=sp)
                                nc.tensor.matmul(ups[:, ns], wu_t[:, kt, :],
                                                 xn2[:, kt, ns],
                                                 start=st, stop=sp)
                        sg = p4s.tile([P, C], BF16, tag="sg", bufs=2)
                        nc.scalar.activation(sg, gps, AF.Silu)
                        nc.vector.tensor_mul(hmlp[:, iq, :], sg, ups)

                    # down proj + (x1/8) residual share -> outT
                    for hm in range(16):
                        dps = p4d.tile([P, C], F32, tag="d")
                        wd_t = p4w.tile([P, 8, P], BF16, tag="wdt", bufs=2)
                        nc.sync.dma_start(
                            wd_t, wd_d[:, P * hm:P * (hm + 1)].rearrange(
                                "(t p) m -> p t m", p=P))
                        for kt8 in range(8):
                            for n in range(2):
                                ns = slice(512 * n, 512 * (n + 1))
                                nc.tensor.matmul(dps[:, ns], wd_t[:, kt8, :],
                                                 hmlp[:, kt8, ns],
                                                 start=(kt8 == 0),
                                                 stop=(kt8 == 7))
                        dsb = p4s.tile([P, C], F32, tag="dsb", bufs=2)
                        nc.vector.scalar_tensor_tensor(
                            dsb, x1[:, hm, :], 1.0 / N_CORES, dps,
                            op0=ALU.mult, op1=ALU.add)
                        nc.sync.dma_start(outT_d[P * hm:P * (hm + 1), cc], dsb)

    nc.compile()
    return nc


_CACHE = {}


def _get_nc():
    if "nc" not in _CACHE:
        _CACHE["nc"] = build()
    return _CACHE["nc"]


def _prep_inputs(inputs):
    """Shard + preprocess full inputs into 8 per-core in_maps."""
    f = lambda k: np.asarray(inputs[k], dtype=np.float32)
    hidden = f("hidden_states")[0]                 # [S, HID]
    sin_t, cos_t = f("sin_table"), f("cos_table")  # [S, 32]
    ln1, ln2 = f("ln1_w"), f("ln2_w")
    wq = f("wq") * ln1[:, None]
    wk = f("wk") * ln1[:, None]
    wv = f("wv") * ln1[:, None]
    wo = f("wo")
    wg = (f("w_gate") * ln2[:, None]).astype(ml_dtypes.bfloat16)
    wu = (f("w_up") * ln2[:, None]).astype(ml_dtypes.bfloat16)
    wd = f("w_down").astype(ml_dtypes.bfloat16)

    hT = np.ascontiguousarray(hidden.T)
    # rows per 64-block: [+sinT (x0 source); -sinT (x1 source)]
    sin4 = np.ascontiguousarray(
        np.tile(np.concatenate([sin_t.T, -sin_t.T], axis=0), (2, 1)))
    cos4 = np.ascontiguousarray(np.tile(cos_t.T, (4, 1)))
    ident = np.eye(P, dtype=np.float32)
    rr = np.arange(P)[:, None]
    cols = np.arange(512)[None, :]
    masks = np.concatenate(
        [(rr + 128 * t <= cols).astype(np.float32) for t in range(4)],
        axis=1)

    in_maps = []
    for c in range(N_CORES):
        qs = slice(QD * c, QD * (c + 1))
        ks = slice(HD * c, HD * (c + 1))
        isl = slice(INTER_LOC * c, INTER_LOC * (c + 1))
        in_maps.append({
            "hT": hT,
            "sin4": sin4,
            "cos4": cos4,
            "wq": np.ascontiguousarray(wq[:, qs]),
            "wkv": np.ascontiguousarray(
                np.concatenate([wk[:, ks], wv[:, ks]], axis=1)),
            "wo": np.ascontiguousarray(wo[qs, :]),
            "wg": np.ascontiguousarray(wg[:, isl]),
            "wu": np.ascontiguousarray(wu[:, isl]),
            "wd": np.ascontiguousarray(wd[isl, :]),
            "ident": ident,
            "masks": masks,
        })
    return in_maps


def kernel(**inputs):
    nc = _get_nc()
    in_maps = _prep_inputs(inputs)
    res = run_bass_kernel_spmd(nc, in_maps, core_ids=list(range(N_CORES)))
    acc = np.zeros((HID, S), dtype=np.float32)
    for c in range(N_CORES):
        acc += res.results[c]["outT"]
    return np.ascontiguousarray(acc.T)[None, :, :]


# revision 23
# speedup vs baseline: 1.0580x; 1.0417x over previous
"""Trainium2 Bass kernel for nn_DecoderLayer_66408784331382.

Single transformer decoder layer (RMSNorm + GQA attention w/ RoPE + RMSNorm +
SwiGLU MLP), tensor-parallel over 8 NeuronCores:

  - per core: 4 of 32 Q heads, 1 of 8 KV heads, 1024 of 8192 MLP inter cols,
    matching row-shards of wo / w_down.
  - all on-device activations are kept transposed ([hid, tok] etc.) so that
    every matmul is transpose-free; the host supplies hidden_states.T.
  - RMS statistics use an ACT Square pass + ones-column matmul (partition
    reduction); softmax denominators come from a ones-column appended to V in
    the PV matmul; per-token scaling uses partition-stride-0 broadcast DMAs.
  - one on-device fp32 AllReduce joins attention output partials before the
    second RMSNorm; the final down-proj partials (+ x1/8 each) are summed on
    the host during unsharding.
  - attention path is float32r (full-rate fp32 matmuls); the MLP runs bf16.

kernel(**inputs) takes the FULL fp32 inputs of reference.setup_inputs() and
returns the FULL [1, 2048, 2048] fp32 output.
"""

import sys

if "/opt/trn_rl_repo" not in sys.path:
    sys.path.insert(0, "/opt/trn_rl_repo")

import numpy as np
import ml_dtypes

import concourse.bass as bass
import concourse.mybir as mybir
import concourse.tile as tile
from concourse import bacc
from concourse.bass_utils import run_bass_kernel_spmd

# ---- problem constants (hardcoded per contract) ----
N_CORES = 8
S = 2048
HID = 2048
HD = 64
NH = 32
INTER = 8192
EPS = 1e-6

QD = (NH // N_CORES) * HD        # 256 local q cols
INTER_LOC = INTER // N_CORES     # 1024
SCALE = 1.0 / np.sqrt(HD)

F32 = mybir.dt.float32
F32R = mybir.dt.float32r
BF16 = mybir.dt.bfloat16

P = 128
Q = 512      # phase-1 token quarter
C = 1024     # phase-4 token chunk
ARDT = mybir.dt.float32  # collective dtype
AF = mybir.ActivationFunctionType
ALU = mybir.AluOpType


def _bcast(ap, parts):
    """View a [1, N] AP as [parts, N] via partition-stride-0 (DMA broadcast)."""
    return bass.AP(tensor=ap.tensor, offset=ap.offset,
                   ap=[[0, parts]] + [list(p) for p in ap.ap[1:]])


def build():
    nc = bacc.Bacc("TRN2", target_bir_lowering=False, debug=False,
                   num_devices=N_CORES)

    hT_d = nc.dram_tensor("hT", [HID, S], F32R, kind="ExternalInput")
    sin4_d = nc.dram_tensor("sin4", [P, S], F32R, kind="ExternalInput")
    cos4_d = nc.dram_tensor("cos4", [P, S], F32R, kind="ExternalInput")
    wq_d = nc.dram_tensor("wq", [HID, QD], F32R, kind="ExternalInput")
    wkv_d = nc.dram_tensor("wkv", [HID, 2 * HD], F32R, kind="ExternalInput")
    wo_d = nc.dram_tensor("wo", [QD, HID], F32R, kind="ExternalInput")
    wg_d = nc.dram_tensor("wg", [HID, INTER_LOC], BF16, kind="ExternalInput")
    wu_d = nc.dram_tensor("wu", [HID, INTER_LOC], BF16, kind="ExternalInput")
    wd_d = nc.dram_tensor("wd", [INTER_LOC, HID], BF16, kind="ExternalInput")
    ident_d = nc.dram_tensor("ident", [P, P], F32R, kind="ExternalInput")
    masks_d = nc.dram_tensor("masks", [P, 4 * 512], F32R, kind="ExternalInput")
    outT_d = nc.dram_tensor("outT", [HID, S], F32, kind="ExternalOutput")

    with tile.TileContext(nc) as tc, nc.allow_low_precision(
            reason="float32r is fp32 bits; reciprocal outputs are fp32-width"):
        with (
            tc.tile_pool(name="const", bufs=1) as const,
            tc.tile_pool(name="dramp", bufs=1, space="DRAM") as dram,
        ):
            ones1 = const.tile([P, 1], F32R)
            eps1 = const.tile([P, 1], F32)
            nc.gpsimd.memset(eps1, EPS)
            # f32r memset fails the walrus ISA check; masks[:,0,511] is all-1.0
            nc.sync.dma_start(
                ones1, bass.AP(tensor=masks_d.tensor
                               if hasattr(masks_d, "tensor") else masks_d,
                               offset=511, ap=[[4 * 512, P], [0, 1]]))

            ar_in = [dram.tile([HID, C], ARDT, name=f"ar_in{i}",
                               tag=f"ar_in{i}") for i in range(2)]
            ar_out = [dram.tile([HID, C], ARDT, addr_space="Shared",
                                name=f"ar_out{i}", tag=f"ar_out{i}")
                      for i in range(2)]
            bc1_dram = dram.tile([4, Q], F32R)
            bc2_dram = dram.tile([2, 8, 512], F32R)
            bc4_dram = dram.tile([2, C], F32R)

            # ======== attention scope (phases 1-3 share these tensors) ======
            with tc.tile_pool(name="keep", bufs=1) as keep:
                sin4 = keep.tile([P, S], F32R)
                cos4 = keep.tile([P, S], F32R)
                ident = keep.tile([P, P], F32R)
                masks = keep.tile([P, 4, 512], F32R)
                nc.sync.dma_start(sin4, sin4_d[:, :])
                nc.sync.dma_start(cos4, cos4_d[:, :])
                nc.sync.dma_start(ident, ident_d[:, :])
                nc.sync.dma_start(
                    masks, masks_d[:, :].rearrange("p (t n) -> p t n", t=4))
                qT = [keep.tile([P, S], F32R, tag=f"qT{m}", name=f"qT{m}") for m in range(2)]
                kTdup = keep.tile([P, S], F32R, tag="kTdup")
                v_ones = keep.tile([P, 16, HD + 1], F32R, tag="v_ones")
                attnT = [keep.tile([P, S], F32R, tag=f"attnT{m}", name=f"attnT{m}")
                         for m in range(2)]
                nc.sync.dma_start(
                    v_ones[:, :, HD:HD + 1],
                    bass.AP(tensor=masks_d.tensor
                            if hasattr(masks_d, "tensor") else masks_d,
                            offset=511, ap=[[4 * 512, P], [0, 16], [0, 1]]))

                # ---- Phase 1: RMS1 + QKV + RoPE, per 512-token quarter ----
                with (
                    tc.tile_pool(name="p1w", bufs=1) as p1w,
                    tc.tile_pool(name="p1x", bufs=1) as p1x,
                    tc.tile_pool(name="p1s", bufs=1) as p1s,
                    tc.tile_pool(name="p1ps", bufs=2, space="PSUM") as p1ps,
                    tc.tile_pool(name="p1ps_s", bufs=1, space="PSUM") as p1pss,
                ):
                    wq_all = p1w.tile([P, 16, QD], F32R)
                    wkv_all = p1w.tile([P, 16, 2 * HD], F32R)
                    nc.sync.dma_start(
                        wq_all, wq_d[:, :].rearrange("(t p) m -> p t m", p=P))
                    nc.sync.dma_start(
                        wkv_all, wkv_d[:, :].rearrange("(t p) m -> p t m", p=P))
                    xn1 = p1x.tile([P, 16, Q], F32R, tag="xn1")

                    for q4 in range(4):
                        qc = slice(Q * q4, Q * (q4 + 1))
                        # RMS1 stats
                        ssq = p1pss.tile([1, Q], F32, tag="ssq")
                        for t4 in range(4):
                            nc.sync.dma_start(
                                xn1[:, 4 * t4:4 * (t4 + 1), :],
                                hT_d[512 * t4:512 * (t4 + 1), qc].rearrange(
                                    "(t p) m -> p t m", p=P))
                        for kt in range(16):
                            xt = xn1[:, kt, :]
                            sq = p1s.tile([P, Q], F32R, tag="sq", bufs=3)
                            nc.scalar.activation(sq, xt, AF.Square)
                            nc.tensor.matmul(ssq, ones1, sq,
                                             start=(kt == 0), stop=(kt == 15))
                        rms = p1s.tile([1, Q], F32R, tag="rms", bufs=2)
                        nc.scalar.activation(rms, ssq, AF.Sqrt,
                                             bias=eps1[0:1, :], scale=1.0 / HID)
                        inv = p1s.tile([1, Q], F32R, tag="inv", bufs=2)
                        nc.vector.reciprocal(inv, rms)
                        invb = p1s.tile([P, Q], F32R, tag="invb", bufs=2)
                        nc.sync.dma_start(bc1_dram[q4:q4 + 1, :], inv)
                        nc.sync.dma_start(invb, _bcast(bc1_dram[q4:q4 + 1, :], P))
                        for kt in range(16):
                            nc.vector.tensor_mul(xn1[:, kt, :],
                                                 xn1[:, kt, :], invb)

                        # QKV projections (transposed outputs)
                        q_ps = [p1ps.tile([P, Q], F32, tag=f"qps{m}", name=f"qps{m}")
                                for m in range(2)]
                        kv_ps = p1ps.tile([P, Q], F32, tag="kvps")
                        for kt in range(16):
                            st, sp = (kt == 0), (kt == 15)
                            for m in range(2):
                                nc.tensor.matmul(
                                    q_ps[m], wq_all[:, kt, P * m:P * (m + 1)],
                                    xn1[:, kt, :], start=st, stop=sp)
                            nc.tensor.matmul(kv_ps, wkv_all[:, kt, :],
                                             xn1[:, kt, :], start=st, stop=sp)

                        # RoPE eviction (sin4 rows carry the rotate-half
                        # sign: +sinT for x0, -sinT for x1 source rows):
                        # out = ps*cos + swap_half(ps)*sinA
                        for m in range(2):
                            s1 = p1s.tile([P, Q], F32R, tag="s1", bufs=2)
                            s2 = p1s.tile([P, Q], F32R, tag="s2", bufs=2)
                            nc.vector.tensor_mul(s1, q_ps[m], cos4[:, qc])
                            for b in range(2):
                                x0 = slice(64 * b, 64 * b + 32)
                                x1s = slice(64 * b + 32, 64 * b + 64)
                                nc.vector.tensor_mul(
                                    s2[x0, :], q_ps[m][x1s, :], sin4[x1s, qc])
                                nc.vector.tensor_mul(
                                    s2[x1s, :], q_ps[m][x0, :], sin4[x0, qc])
                            nc.vector.tensor_add(qT[m][:, qc], s1, s2)
                        # RoPE eviction: k, duplicated into rows 64:128
                        s1 = p1s.tile([64, Q], F32R, tag="s1k", bufs=2)
                        s2 = p1s.tile([64, Q], F32R, tag="s2k", bufs=2)
                        nc.vector.tensor_mul(s1, kv_ps[0:64, :], cos4[0:64, qc])
                        nc.vector.tensor_mul(
                            s2[0:32, :], kv_ps[32:64, :], sin4[32:64, qc])
                        nc.vector.tensor_mul(
                            s2[32:64, :], kv_ps[0:32, :], sin4[0:32, qc])
                        nc.vector.tensor_add(kTdup[0:64, qc], s1, s2)
                        nc.vector.tensor_copy(kTdup[64:128, qc], kTdup[0:64, qc])
                        # v: vT then PE-transpose into v_ones
                        vt = p1s.tile([64, Q], F32R, tag="vt", bufs=2)
                        nc.vector.tensor_copy(vt, kv_ps[64:128, :])
                        for j in range(4):
                            ktg = 4 * q4 + j
                            vtp = p1pss.tile([P, HD], F32R, tag="vtp")
                            nc.tensor.transpose(
                                vtp, vt[:, P * j:P * (j + 1)],
                                ident[0:64, 0:64])
                            nc.vector.tensor_copy(v_ones[:, ktg, 0:HD], vtp)

                # ---- Phases 2+3 interleaved per token half: attention
                #      for half h, o-proj for half h, AllReduce(h).  The
                #      attention work of half 1 overlaps AllReduce(0). ----
                with (
                    tc.tile_pool(name="p2pr", bufs=3) as p2pr,
                    tc.tile_pool(name="p2sm", bufs=2) as p2sm,
                    tc.tile_pool(name="p3w", bufs=1) as p3w,
                    tc.tile_pool(name="p3o", bufs=3) as p3o,
                    tc.tile_pool(name="p2ps", bufs=2, space="PSUM") as p2ps,
                    tc.tile_pool(name="p2pv", bufs=1, space="PSUM") as p2pv,
                    tc.tile_pool(name="p3ps", bufs=2, space="PSUM") as p3ps,
                ):
                    wo_all = p3w.tile([P, 2, HID], F32R)
                    nc.sync.dma_start(
                        wo_all, wo_d[:, :].rearrange("(t p) m -> p t m", p=P))
                    for c3 in range(2):
                        for qc4 in range(2 * c3, 2 * c3 + 2):
                            for m in range(2):
                                qs = slice(512 * qc4, 512 * (qc4 + 1))
                                pv = [p2pv.tile([HD + 1, 512], F32,
                                                tag=f"pv{b}", name=f"pv{b}")
                                      for b in range(2)]
                                nkt = 4 * qc4 + 4
                                for kt in range(nkt):
                                    st, sp = (kt == 0), (kt == nkt - 1)
                                    for b in range(2):
                                        rows = slice(64 * b, 64 * (b + 1))
                                        sc = p2ps.tile([P, 512], F32,
                                                       tag=f"sc{b}")
                                        nc.tensor.matmul(
                                            sc,
                                            kTdup[rows, P * kt:P * (kt + 1)],
                                            qT[m][rows, qs],
                                            start=True, stop=True)
                                        pr = p2pr.tile([P, 512], F32R,
                                                       tag=f"pr{b}")
                                        nc.scalar.activation(
                                            pr, sc, AF.Exp, scale=float(SCALE))
                                        if kt >= 4 * qc4:
                                            nc.vector.tensor_mul(
                                                pr, pr,
                                                masks[:, kt - 4 * qc4, :])
                                        nc.tensor.matmul(
                                            pv[b], v_ones[:, kt, :], pr,
                                            start=st, stop=sp)
                                for b in range(2):
                                    rec = p2sm.tile([1, 512], F32R,
                                                    tag=f"rec{b}")
                                    nc.vector.reciprocal(
                                        rec, pv[b][HD:HD + 1, :])
                                    slot = bc2_dram[b:b + 1, 4 * m + qc4, :]
                                    nc.sync.dma_start(slot, rec)
                                    recb = p2sm.tile([64, 512], F32R,
                                                     tag=f"recb{b}")
                                    nc.sync.dma_start(recb, _bcast(slot, 64))
                                    nc.vector.tensor_mul(
                                        attnT[m][64 * b:64 * (b + 1), qs],
                                        pv[b][0:HD, :], recb)
                        # o-proj for this half -> ar_in[c3]
                        for hm in range(16):
                            osb = p3o.tile([P, C], ARDT, tag="osb")
                            for nq in range(2):
                                qc4 = 2 * c3 + nq
                                qs = slice(512 * qc4, 512 * (qc4 + 1))
                                ops = p3ps.tile([P, 512], F32, tag="ops")
                                for kt2 in range(2):
                                    nc.tensor.matmul(
                                        ops,
                                        wo_all[:, kt2, P * hm:P * (hm + 1)],
                                        attnT[kt2][:, qs],
                                        start=(kt2 == 0), stop=(kt2 == 1))
                                nc.scalar.copy(
                                    osb[:, 512 * nq:512 * (nq + 1)], ops)
                            nc.sync.dma_start(
                                ar_in[c3][P * hm:P * (hm + 1), :], osb)
                        # AllReduce for this token half (overlaps what follows)
                        nc.gpsimd.collective_compute(
                            "AllReduce", ALU.add,
                            replica_groups=[list(range(N_CORES))],
                            ins=[ar_in[c3][:, :].opt()],
                            outs=[ar_out[c3][:, :].opt()])

            # ---- Phase 4: x1 + RMS2 + SwiGLU MLP, per 1024-token chunk ----
            with (
                tc.tile_pool(name="p4x", bufs=1) as p4x,
                tc.tile_pool(name="p4s", bufs=1) as p4s,
                tc.tile_pool(name="p4w", bufs=1) as p4w,
                tc.tile_pool(name="p4ps_s", bufs=1, space="PSUM") as p4pss,
                tc.tile_pool(name="p4ps_gu", bufs=1, space="PSUM") as p4gu,
                tc.tile_pool(name="p4ps_d", bufs=1, space="PSUM") as p4d,
            ):
                x1 = p4x.tile([P, 16, C], F32R, tag="x1")
                xn2 = p4x.tile([P, 16, C], BF16, tag="xn2")
                hmlp = p4x.tile([P, 8, C], BF16, tag="hmlp")
                for c2 in range(2):
                    cc = slice(C * c2, C * (c2 + 1))
                    # x1 = hidden + attn_out ; RMS2 stats
                    ssq2 = p4pss.tile([1, C], F32, tag="ssq2")
                    for kt in range(16):
                        rs = slice(P * kt, P * (kt + 1))
                        th = p4s.tile([P, C], F32R, tag="th", bufs=2)
                        ta = p4s.tile([P, C], ARDT, tag="ta", bufs=2)
                        nc.sync.dma_start(th, hT_d[rs, cc])
                        nc.sync.dma_start(ta, ar_out[c2][rs, :])
                        nc.vector.tensor_add(x1[:, kt, :], th, ta)
                        sq = p4s.tile([P, C], F32R, tag="sq2", bufs=2)
                        nc.scalar.activation(sq, x1[:, kt, :], AF.Square)
                        for n in range(2):
                            nc.tensor.matmul(
                                ssq2[:, 512 * n:512 * (n + 1)], ones1,
                                sq[:, 512 * n:512 * (n + 1)],
                                start=(kt == 0), stop=(kt == 15))
                    rms = p4s.tile([1, C], F32R, tag="rms2", bufs=2)
                    nc.scalar.activation(rms, ssq2, AF.Sqrt,
                                         bias=eps1[0:1, :], scale=1.0 / HID)
                    inv = p4s.tile([1, C], F32R, tag="inv2", bufs=2)
                    nc.vector.reciprocal(inv, rms)
                    invb = p4s.tile([P, C], F32R, tag="invb2", bufs=1)
                    nc.sync.dma_start(bc4_dram[c2:c2 + 1, :], inv)
                    nc.sync.dma_start(invb, _bcast(bc4_dram[c2:c2 + 1, :], P))
                    for kt in range(16):
                        nc.vector.tensor_mul(xn2[:, kt, :], x1[:, kt, :], invb)

                    # gate/up + silu*up (bf16)
                    for iq in range(8):
                        gps = p4gu.tile([P, C], F32, tag="g")
                        ups = p4gu.tile([P, C], F32, tag="u")
                        wg_t = p4w.tile([P, 16, P], BF16, tag="wgt", bufs=2)
                        wu_t = p4w.tile([P, 16, P], BF16, tag="wut", bufs=2)
                        nc.sync.dma_start(
                            wg_t, wg_d[:, P * iq:P * (iq + 1)].rearrange(
                                "(t p) m -> p t m", p=P))
                        nc.sync.dma_start(
                            wu_t, wu_d[:, P * iq:P * (iq + 1)].rearrange(
                                "(t p) m -> p t m", p=P))
                        for kt in range(16):
                            st, sp = (kt == 0), (kt == 15)
                            for n in range(2):
                                ns = slice(512 * n, 512 * (n + 1))
                                nc.tensor.matmul(gps[:, ns], wg_t[:, kt, :],
                                                 xn2[:, kt, ns],
                                                 start=st, stop=sp)
                                nc.tensor.matmul(ups[:, ns], wu_t[:, kt, :],
                                                 xn2[:, kt, ns],
                                                 start=st, stop=sp)
                        sg = p4s.tile([P, C], BF16, tag="sg", bufs=2)
                        nc.scalar.activation(sg, gps, AF.Silu)
                        nc.vector.tensor_mul(hmlp[:, iq, :], sg, ups)

                    # down proj + (x1/8) residual share -> outT
                    for hm in range(16):
                        dps = p4d.tile([P, C], F32, tag="d")
                        wd_t = p4w.tile([P, 8, P], BF16, tag="wdt", bufs=2)
                        nc.sync.dma_start(
                            wd_t, wd_d[:, P * hm:P * (hm + 1)].rearrange(
                                "(t p) m -> p t m", p=P))
                        for kt8 in range(8):
                            for n in range(2):
                                ns = slice(512 * n, 512 * (n + 1))
                                nc.tensor.matmul(dps[:, ns], wd_t[:, kt8, :],
                                                 hmlp[:, kt8, ns],
                                                 start=(kt8 == 0),
                                                 stop=(kt8 == 7))
                        dsb = p4s.tile([P, C], F32, tag="dsb", bufs=2)
                        nc.vector.scalar_tensor_tensor(
                            dsb, x1[:, hm, :], 1.0 / N_CORES, dps,
                            op0=ALU.mult, op1=ALU.add)
                        nc.sync.dma_start(outT_d[P * hm:P * (hm + 1), cc], dsb)

    nc.compile()
    return nc


_CACHE = {}


def _get_nc():
    if "nc" not in _CACHE:
        _CACHE["nc"] = build()
    return _CACHE["nc"]


def _prep_inputs(inputs):
    """Shard + preprocess full inputs into 8 per-core in_maps."""
    f = lambda k: np.asarray(inputs[k], dtype=np.float32)
    hidden = f("hidden_states")[0]                 # [S, HID]
    sin_t, cos_t = f("sin_table"), f("cos_table")  # [S, 32]
    ln1, ln2 = f("ln1_w"), f("ln2_w")
    wq = f("wq") * ln1[:, None]
    wk = f("wk") * ln1[:, None]
    wv = f("wv") * ln1[:, None]
    wo = f("wo")
    wg = (f("w_gate") * ln2[:, None]).astype(ml_dtypes.bfloat16)
    wu = (f("w_up") * ln2[:, None]).astype(ml_dtypes.bfloat16)
    wd = f("w_down").astype(ml_dtypes.bfloat16)

    hT = np.ascontiguousarray(hidden.T)
    # rows per 64-block: [+sinT (x0 source); -sinT (x1 source)]
    sin4 = np.ascontiguousarray(
        np.tile(np.concatenate([sin_t.T, -sin_t.T], axis=0), (2, 1)))
    cos4 = np.ascontiguousarray(np.tile(cos_t.T, (4, 1)))
    ident = np.eye(P, dtype=np.float32)
    rr = np.arange(P)[:, None]
    cols = np.arange(512)[None, :]
    masks = np.concatenate(
        [(rr + 128 * t <= cols).astype(np.float32) for t in range(4)],
        axis=1)

    in_maps = []
    for c in range(N_CORES):
        qs = slice(QD * c, QD * (c + 1))
        ks = slice(HD * c, HD * (c + 1))
        isl = slice(INTER_LOC * c, INTER_LOC * (c + 1))
        in_maps.append({
            "hT": hT,
            "sin4": sin4,
            "cos4": cos4,
            "wq": np.ascontiguousarray(wq[:, qs]),
            "wkv": np.ascontiguousarray(
                np.concatenate([wk[:, ks], wv[:, ks]], axis=1)),
            "wo": np.ascontiguousarray(wo[qs, :]),
            "wg": np.ascontiguousarray(wg[:, isl]),
            "wu": np.ascontiguousarray(wu[:, isl]),
            "wd": np.ascontiguousarray(wd[isl, :]),
            "ident": ident,
            "masks": masks,
        })
    return in_maps


def kernel(**inputs):
    nc = _get_nc()
    in_maps = _prep_inputs(inputs)
    res = run_bass_kernel_spmd(nc, in_maps, core_ids=list(range(N_CORES)))
    acc = np.zeros((HID, S), dtype=np.float32)
    for c in range(N_CORES):
        acc += res.results[c]["outT"]
    return np.ascontiguousarray(acc.T)[None, :, :]
